# revision 1
# baseline (speedup 1.0000x reference)
"""Trainium2 Bass kernel for nn_FragAttention (segment_reduce).

Reference computation (S=128, B=512, D=512, G=S-1=127):
    xb     = transpose(x, (1,0,2))            # (B, S, D)
    xm     = xb * (~src_mask)[:, :, None]     # zero padded tokens
    left   [b,g,d] = sum_{s<=g} xm[b,s,d]     # masked prefix sums
    right  [b,g,d] = sum_{s>g}  xm[b,s,d]
    out    = concat([left, right], axis=2)    # (B, G, 2D)

Strategy: pure data parallel over B across 8 cores (64 batches each).
The pad mask is folded into x on the host (exact: multiply by 0/1).
Per batch the prefix/suffix sums are computed on the TensorEngine as two
matmuls against constant 0/1 triangular matrices (contraction over S=128
on partitions), written to adjacent PSUM banks so a single DVE copy per
batch assembles the (G, 2D) output block. float32r (same 4-byte storage
as f32, reduced-precision multiply at 4x the f32 matmul rate) is used
for the matmul inputs; the triangular weights are exact 0/1 so only x's
mantissa truncation matters (~1e-4 rel err).
"""

import numpy as np

import concourse.bass as bass
import concourse.mybir as mybir
from concourse import bacc
from concourse.tile import TileContext
from concourse.bass_utils import run_bass_kernel_spmd

S, B, D = 128, 512, 512
G = S - 1
N_CORES = 8
BL = B // N_CORES  # 64 batches per core

IN_CHUNK = 8   # batches per input DMA
OUT_CHUNK = 2  # batches per output DMA

_NC_CACHE = None


def _build_bass(repeats: int = 1) -> bass.Bass:
    """repeats>1 re-runs the whole body (same I/O) — timing calibration only."""
    nc = bacc.Bacc(dynamic_dma_scratch_size=32768)
    f32 = mybir.dt.float32
    f32r = mybir.dt.float32r

    x_in = nc.declare_dram_parameter("x", [S, BL, D], f32r, isOutput=False)
    # tri[:, 0:128] = upper (incl diag)  tri[s,g] = 1 if s <= g  -> prefix sums
    # tri[:, 128:256] = strictly lower   tri[s,g] = 1 if s >  g  -> suffix sums
    t_in = nc.declare_dram_parameter("tri", [S, 2 * S], f32r, isOutput=False)
    # g-major per-core output: one partition row g maps to a 256KB contiguous
    # DRAM run, so output DMA descriptors are 32KB+ (4KB-granular descriptors
    # on the single dynamic HWDGE ring were the bottleneck: ~140ns each).
    # The host transposes (G, BL, 2D) -> (BL, G, 2D) while gathering.
    out = nc.declare_dram_parameter("out", [G, BL, 2 * D], f32, isOutput=True)

    with TileContext(nc) as tc:
        with (
            tc.tile_pool(name="const", bufs=1) as cpool,
            tc.tile_pool(name="xin", bufs=2) as xpool,
            tc.tile_pool(name="outs", bufs=14) as opool,
            tc.tile_pool(name="psum", bufs=2, space="PSUM") as ppool,
        ):
            trir = cpool.tile([S, 2 * S], f32r)
            nc.sync.dma_start(out=trir[:], in_=t_in[:])
            ut = trir[:, 0:S]        # (128, 128) stationary, left sums
            lt = trir[:, S : 2 * S]  # (128, 128) stationary, right sums

            BLK = max(IN_CHUNK, OUT_CHUNK)

            def per_pair(xt, ot, j, k):
                """2 batches (j, j+1) of xt -> slots (k, k+1) of ot.

                One 4-bank PSUM tile takes all 4 matmuls; DVE copies batch j,
                ACT copies batch j+1 — one copy op per batch total, halving
                the cross-engine semaphore edges vs per-batch tiles.
                """
                ps = ppool.tile([S, 4, D], f32)  # 4 adjacent banks
                for h, (b, tri_) in enumerate(
                    [(j, ut), (j, lt), (j + 1, ut), (j + 1, lt)]
                ):
                    nc.tensor.matmul(out=ps[:, h, :], lhsT=tri_, rhs=xt[:, b, :],
                                     start=True, stop=True)
                nc.vector.tensor_copy(
                    out=ot[0:G, k, :].rearrange("g (h d) -> g h d", h=2),
                    in_=ps[0:G, 0:2, :],
                )
                nc.scalar.activation(
                    out=ot[0:G, k + 1, :].rearrange("g (h d) -> g h d", h=2),
                    in_=ps[0:G, 2:4, :],
                    func=mybir.ActivationFunctionType.Copy,
                )

            def out_dma(ot, o0):
                # SWDGE (gpsimd): dynamic HBM writes on the HWDGE rings all
                # serialize on one SDMA engine (~27 GB/s); SWDGE assigns each
                # DMA a dedicated engine (~22.6 GB/s each) and Tile's 8 DMASW
                # sem lanes keep ~7 in flight -> ~150 GB/s write throughput.
                # (Tried and rejected: sync/scalar-ring write shares block
                # other traffic; c-half splits -> 609us; OUT_CHUNK 1/4/8/16
                # all slower; bigger SWDGE ring no effect.)
                if o0 >= BL - 4:
                    # tail: the kernel ends waiting for the last DMA's
                    # single-engine drain (~46us/MB) — issue the final
                    # batches as single-batch DMAs to halve that tail.
                    # (Also tried: the same split on every 3rd chunk to
                    # desynchronize the DMASW lane limit cycle -> 375us,
                    # worse; the extra lane-cycles per byte dominate.)
                    for t in range(OUT_CHUNK):
                        nc.gpsimd.dma_start(
                            out=out[:, o0 + t : o0 + t + 1, :],
                            in_=ot[0:G, t : t + 1, :],
                        )
                else:
                    nc.gpsimd.dma_start(
                        out=out[:, o0 : o0 + OUT_CHUNK, :], in_=ot[0:G, :, :],
                    )

            for b0 in [b for _ in range(repeats) for b in range(0, BL, BLK)]:
                if OUT_CHUNK >= IN_CHUNK:
                    ot = opool.tile([S, OUT_CHUNK, 2 * D], f32)
                    for c0 in range(b0, b0 + OUT_CHUNK, IN_CHUNK):
                        xt = xpool.tile([S, IN_CHUNK, D], f32r)
                        nc.sync.dma_start(
                            out=xt[:], in_=x_in[:, c0 : c0 + IN_CHUNK, :])
                        for j in range(0, IN_CHUNK, 2):
                            per_pair(xt, ot, j, c0 - b0 + j)
                    out_dma(ot, b0)
                else:
                    xt = xpool.tile([S, IN_CHUNK, D], f32r)
                    nc.sync.dma_start(
                        out=xt[:], in_=x_in[:, b0 : b0 + IN_CHUNK, :])
                    for o0 in range(b0, b0 + IN_CHUNK, OUT_CHUNK):
                        ot = opool.tile([S, OUT_CHUNK, 2 * D], f32)
                        for j in range(0, OUT_CHUNK, 2):
                            per_pair(xt, ot, o0 - b0 + j, j)
                        out_dma(ot, o0)
    nc.finalize()  # runs the Bacc pass pipeline (reg alloc, wait splitting)
    return nc


def _get_nc() -> bass.Bass:
    global _NC_CACHE
    if _NC_CACHE is None:
        _NC_CACHE = _build_bass()
    return _NC_CACHE


def _make_in_maps(x: np.ndarray, src_mask: np.ndarray) -> list[dict]:
    x = np.asarray(x, dtype=np.float32)
    src_mask = np.asarray(src_mask)
    assert x.shape == (S, B, D), x.shape
    assert src_mask.shape == (B, S), src_mask.shape

    valid = (~src_mask.astype(bool)).astype(np.float32).T  # (S, B)
    xm = x * valid[:, :, None]  # exact: zero out padded tokens on host
    tri = np.concatenate(
        [
            np.triu(np.ones((S, S), np.float32)),       # s <= g
            np.tril(np.ones((S, S), np.float32), -1),   # s >  g
        ],
        axis=1,
    )

    in_maps = []
    for i in range(N_CORES):
        sl = slice(i * BL, (i + 1) * BL)
        in_maps.append(
            {
                "x": np.ascontiguousarray(xm[:, sl, :]),
                "tri": tri,
            }
        )
    return in_maps


def _assemble(results: list[dict]) -> np.ndarray:
    full = np.empty((B, G, 2 * D), dtype=np.float32)
    for i in range(N_CORES):
        full[i * BL : (i + 1) * BL] = results[i]["out"].transpose(1, 0, 2)
    return full


def kernel(x: np.ndarray, src_mask: np.ndarray) -> np.ndarray:
    in_maps = _make_in_maps(x, src_mask)
    res = run_bass_kernel_spmd(_get_nc(), in_maps, core_ids=list(range(N_CORES)))
    return _assemble(res.results)

